# revision 18
# baseline (speedup 1.0000x reference)
"""Trainium2 Bass kernel for nn_AttentionAggregator (segment_reduce).

Math: out[b, g] = sum_{j in group g} softmax_g(att)[j] * feat[b, flat_idx[j]]
    = (feat @ W)[b, g]   with W[k, g] = sum_{j in g, flat_idx[j] = k} attn[j]

The segment softmax and the scatter that builds W involve only the tiny
index/weight tensors, so they run on host.  The heavy part — the
(4096 x 4096) @ (4096 x 1024) product — runs on 8 NeuronCores with the
batch axis sharded 512 rows per core (embarrassingly parallel, no
collectives).  Each core computes outT = W^T-blocks x featT via 256
accumulating fp16 matmuls (lhsT = W k/g-tile, rhs = featT k-tile; fp32
psum).  fp16 operands halve DMA vs fp32 and carry ~5.6e-4 rel absmax
error (8x better than bf16 for this data).

Self-contained: hardcodes shapes from the problem spec; no sibling imports.
"""

import numpy as np

B = 4096
NG = 4096
G = 1024
N_CORES = 8
B_LOC = B // N_CORES          # 512 batch rows per core
P = 128                       # SBUF/PE partitions
KT = NG // P                  # 32 contraction tiles
GT = G // P                   # 8 output-group tiles

_NC_CACHE = {}


def _host_softmax_scatter(att_weights, flat_idx, segment_ids, num_segments):
    """Per-segment softmax of att_weights, scatter-added into dense W (NG, G)."""
    aw = np.asarray(att_weights, dtype=np.float32)
    seg = np.asarray(segment_ids, dtype=np.int64)
    idx = np.asarray(flat_idx, dtype=np.int64)
    n_seg = int(num_segments)

    seg_max = np.full(n_seg, -np.inf, dtype=np.float32)
    np.maximum.at(seg_max, seg, aw)
    ex = np.exp(aw - seg_max[seg])
    denom = np.zeros(n_seg, dtype=np.float32)
    np.add.at(denom, seg, ex)
    attn = ex / denom[seg]

    w = np.zeros((NG, n_seg), dtype=np.float32)
    np.add.at(w, (idx, seg), attn)
    return w


def build_nc():
    """SPMD single-core program: outT(1024, 512) = W(4096, 1024)^T @ featT(4096, 512)."""
    import concourse.mybir as mybir
    from concourse import bacc
    from concourse.tile import TileContext

    F32 = mybir.dt.float32
    F16 = mybir.dt.float16

    nc = bacc.Bacc("TRN2", target_bir_lowering=False, debug=False)
    # Host pre-tiles BOTH operands into ONE fused, k-major tensor:
    #   wf[p, k, 0:G]        = W[k*128 + p, g]
    #   wf[p, k, G:G+B_LOC]  = feat[core_b0 + b, k*128 + p]
    # One DMA ring then delivers W and feat for each k-tile in exact
    # consumption order (FIFO per queue), which two separate rings cannot
    # guarantee: SDMA round-robin let the feat ring surge 4 k-tiles ahead
    # while the W ring starved at 61 GB/s, stalling the PE ~3us.  Fused
    # chunks are also 3-12KB contiguous per partition (vs 1-2KB) -> ~300GB/s.
    WF = G + B_LOC
    wf_t = nc.dram_tensor("wf", [P, KT, WF], F16, kind="ExternalInput")
    out_t = nc.dram_tensor("outT", [G, B_LOC], F32, kind="ExternalOutput")

    # k-major lockstep: W and feat stream together in k-chunks; all 8 psum
    # banks accumulate their g-tile simultaneously, so the PE starts as soon
    # as the first small chunk lands and never waits for a full-tensor load.
    # Small chunks first (fast ramp: k0 lands ~2.5us after DMA start), then
    # 4-ktile chunks whose 4-8KB-per-partition descriptors run the rings at
    # full rate.  The last two 4-ktile chunks stay live for the gt-major tail.
    CHUNKS = [1, 1, 1, 1, 2, 2, 4, 4, 4, 4, 4, 4]   # sum = 32 k-tiles
    K_TAIL = 8                          # final k-tiles run gt-major (below)
    # Dummy matmuls must bridge the PE from its first instruction (~7.4us)
    # all the way to data-ready (~10.9us: k0 DMA last-byte ~10.2 + ~0.7us
    # completion-receipt before the sem fires).  An idle gap here resets the
    # HAM activity window and the real matmuls run at 1.2 GHz for ~3.4us
    # (measured +4.4us wall), so round UP: one extra dummy costs only 106ns.
    DUMMIES = 26

    with TileContext(nc) as tc:
        with (
            tc.tile_pool(name="wfp", bufs=8) as wfp,
            tc.tile_pool(name="pp", bufs=8, space="PSUM") as pp,
            tc.tile_pool(name="op", bufs=2) as op,
        ):
            # PE warm-up: the HAM clock gate keeps PE at 1.2 GHz until it has
            # been busy ~3.4us.  Run dummy matmuls while the first DMAs are in
            # flight so the real matmuls start at 2.4 GHz.  memset on GpSimd:
            # its preamble finishes ~0.4us before Vector's, so the first
            # dummy LDWEIGHTS issues that much earlier.
            dummy = op.tile([P, P], F16, tag="dummy", bufs=1)
            nc.gpsimd.memset(dummy, 0)
            ps_all = [
                pp.tile([P, B_LOC], mybir.dt.float32, name=f"ps{gt}", tag="ps")
                for gt in range(GT)
            ]
            for _ in range(DUMMIES):
                nc.tensor.matmul(
                    ps_all[GT - 1][:, :P], lhsT=dummy, rhs=dummy,
                    start=True, stop=True,
                )

            # stream all chunks; keep tiles of the final K_TAIL k-tiles live
            tail_tiles = []
            k0 = 0
            for ci, ck in enumerate(CHUNKS):
                wf_sb = wfp.tile([P, ck, WF], F16, tag="wf", padded_shape=[P, 4, WF])
                if ci == 0:
                    # Split chunk 0 so the first matmuls (gt0..3) wait only
                    # on feat k0 + the first half of W k0 (256KB, ready
                    # ~0.3us before the full 384KB chunk); gt4..7 wait on
                    # the rest.  Sub-DMAs keep >=1KB/partition descriptors.
                    nc.sync.dma_start(wf_sb[:, :, G:], wf_t[:, 0:ck, G:])
                    nc.sync.dma_start(wf_sb[:, :, 0 : G // 2], wf_t[:, 0:ck, 0 : G // 2])
                    nc.sync.dma_start(wf_sb[:, :, G // 2 : G], wf_t[:, 0:ck, G // 2 : G])
                else:
                    nc.sync.dma_start(wf_sb, wf_t[:, k0 : k0 + ck, :])
                if k0 >= KT - K_TAIL:
                    tail_tiles.append((k0, wf_sb))
                    k0 += ck
                    continue
                for kl in range(ck):
                    k = k0 + kl
                    for gt in range(GT):
                        nc.tensor.matmul(
                            ps_all[gt],
                            lhsT=wf_sb[:, kl, gt * P : (gt + 1) * P],
                            rhs=wf_sb[:, kl, G : G + B_LOC],
                            start=(k == 0),
                            stop=False,
                        )
                k0 += ck

            # final K_TAIL k-tiles run gt-major, so psums complete staggered
            # and each copy+store overlaps the remaining gt's matmuls.  All
            # copies ride Vector (idle otherwise); out-DMAs ride the Scalar
            # ring so they never queue behind the input stream on Sync.
            for gt in range(GT):
                for k0, wf_sb in tail_tiles:
                    for kl in range(wf_sb.shape[1]):
                        k = k0 + kl
                        nc.tensor.matmul(
                            ps_all[gt],
                            lhsT=wf_sb[:, kl, gt * P : (gt + 1) * P],
                            rhs=wf_sb[:, kl, G : G + B_LOC],
                            start=False,
                            stop=(k == KT - 1),
                        )
                if gt < GT - 1:
                    o_sb = op.tile([P, B_LOC], F32, tag="o")
                    nc.vector.tensor_copy(o_sb, ps_all[gt])
                    nc.scalar.dma_start(out_t[gt * P : (gt + 1) * P, :], o_sb)
                else:
                    # Last gt is the critical path.  Two 256-col chunks let
                    # copy c1 overlap DMA c0's issue+descriptor pipeline:
                    # measured lastMM->transfer-end 3.0us vs 3.2-3.5us for a
                    # single copy+DMA.
                    for c0 in range(0, B_LOC, 256):
                        o_cb = op.tile([P, 256], F32, tag="ocb", bufs=2)
                        nc.vector.tensor_copy(o_cb, ps_all[gt][:, c0 : c0 + 256])
                        nc.scalar.dma_start(
                            out_t[gt * P : (gt + 1) * P, c0 : c0 + 256], o_cb
                        )
    nc.compile()
    return nc


def make_in_maps(gene_set_features, wmat):
    feat = np.asarray(gene_set_features, dtype=np.float32).astype(np.float16)
    # (P, KT, G): wmat_tiled[p, k, g] = W[k*128 + p, g]
    w_tiled = wmat.astype(np.float16).reshape(KT, P, G).transpose(1, 0, 2)
    in_maps = []
    for c in range(N_CORES):
        shard = feat[c * B_LOC : (c + 1) * B_LOC, :]  # (B_LOC, NG)
        # (P, KT, B_LOC): feat_tiled[p, k, b] = shard[b, k*128 + p]
        feat_tiled = shard.T.reshape(KT, P, B_LOC).transpose(1, 0, 2)
        # fused (P, KT, G + B_LOC): W block then feat block per k-tile, so a
        # single in-order DMA stream supplies both operands k-lockstep.
        wf = np.concatenate([w_tiled, feat_tiled], axis=2)
        in_maps.append({"wf": np.ascontiguousarray(wf)})
    return in_maps


def kernel(gene_set_features, att_weights, flat_idx, segment_ids, num_segments):
    from concourse.bass_utils import run_bass_kernel_spmd

    wmat = _host_softmax_scatter(att_weights, flat_idx, segment_ids, num_segments)
    in_maps = make_in_maps(gene_set_features, wmat)

    if "nc" not in _NC_CACHE:
        _NC_CACHE["nc"] = build_nc()
    nc = _NC_CACHE["nc"]

    res = run_bass_kernel_spmd(nc, in_maps, core_ids=list(range(N_CORES)))

    out = np.empty((B, G), dtype=np.float32)
    for c in range(N_CORES):
        out[c * B_LOC : (c + 1) * B_LOC, :] = res.results[c]["outT"].T
    return out



# revision 19
# speedup vs baseline: 1.0029x; 1.0029x over previous
"""Trainium2 Bass kernel for nn_AttentionAggregator (segment_reduce).

Math: out[b, g] = sum_{j in group g} softmax_g(att)[j] * feat[b, flat_idx[j]]
    = (feat @ W)[b, g]   with W[k, g] = sum_{j in g, flat_idx[j] = k} attn[j]

The segment softmax and the scatter that builds W involve only the tiny
index/weight tensors, so they run on host.  The heavy part — the
(4096 x 4096) @ (4096 x 1024) product — runs on 8 NeuronCores with the
batch axis sharded 512 rows per core (embarrassingly parallel, no
collectives).  Each core computes outT = W^T-blocks x featT via 256
accumulating fp16 matmuls (lhsT = W k/g-tile, rhs = featT k-tile; fp32
psum).  fp16 operands halve DMA vs fp32 and carry ~5.6e-4 rel absmax
error (8x better than bf16 for this data).

Self-contained: hardcodes shapes from the problem spec; no sibling imports.
"""

import numpy as np

B = 4096
NG = 4096
G = 1024
N_CORES = 8
B_LOC = B // N_CORES          # 512 batch rows per core
P = 128                       # SBUF/PE partitions
KT = NG // P                  # 32 contraction tiles
GT = G // P                   # 8 output-group tiles

_NC_CACHE = {}


def _host_softmax_scatter(att_weights, flat_idx, segment_ids, num_segments):
    """Per-segment softmax of att_weights, scatter-added into dense W (NG, G)."""
    aw = np.asarray(att_weights, dtype=np.float32)
    seg = np.asarray(segment_ids, dtype=np.int64)
    idx = np.asarray(flat_idx, dtype=np.int64)
    n_seg = int(num_segments)

    seg_max = np.full(n_seg, -np.inf, dtype=np.float32)
    np.maximum.at(seg_max, seg, aw)
    ex = np.exp(aw - seg_max[seg])
    denom = np.zeros(n_seg, dtype=np.float32)
    np.add.at(denom, seg, ex)
    attn = ex / denom[seg]

    w = np.zeros((NG, n_seg), dtype=np.float32)
    np.add.at(w, (idx, seg), attn)
    return w


def build_nc():
    """SPMD single-core program: outT(1024, 512) = W(4096, 1024)^T @ featT(4096, 512)."""
    import concourse.mybir as mybir
    from concourse import bacc
    from concourse.tile import TileContext

    F32 = mybir.dt.float32
    F16 = mybir.dt.float16

    nc = bacc.Bacc("TRN2", target_bir_lowering=False, debug=False)
    # Host pre-tiles BOTH operands into ONE fused, k-major tensor:
    #   wf[p, k, 0:G]        = W[k*128 + p, g]
    #   wf[p, k, G:G+B_LOC]  = feat[core_b0 + b, k*128 + p]
    # One DMA ring then delivers W and feat for each k-tile in exact
    # consumption order (FIFO per queue), which two separate rings cannot
    # guarantee: SDMA round-robin let the feat ring surge 4 k-tiles ahead
    # while the W ring starved at 61 GB/s, stalling the PE ~3us.  Fused
    # chunks are also 3-12KB contiguous per partition (vs 1-2KB) -> ~300GB/s.
    WF = G + B_LOC
    wf_t = nc.dram_tensor("wf", [P, KT, WF], F16, kind="ExternalInput")
    out_t = nc.dram_tensor("outT", [G, B_LOC], F32, kind="ExternalOutput")

    # k-major lockstep: W and feat stream together in k-chunks; all 8 psum
    # banks accumulate their g-tile simultaneously, so the PE starts as soon
    # as the first small chunk lands and never waits for a full-tensor load.
    # Small chunks first (fast ramp: k0 lands ~2.5us after DMA start), then
    # 4-ktile chunks whose 4-8KB-per-partition descriptors run the rings at
    # full rate.  The last two 4-ktile chunks stay live for the gt-major tail.
    # Finer 2-ktile chunks through the ramp: on slow-ring runs (317 GB/s)
    # the first 4-ktile chunk's completion lagged the PE by ~0.6us at k6;
    # 2-ktile granularity unblocks each pair as it lands.
    CHUNKS = [1, 1, 1, 1, 2, 2, 2, 2, 2, 2, 4, 4, 4, 4]   # sum = 32 k-tiles
    K_TAIL = 8                          # final k-tiles run gt-major (below)
    # Dummy matmuls must bridge the PE from its first instruction (~7.4us)
    # all the way to data-ready (~10.9us: k0 DMA last-byte ~10.2 + ~0.7us
    # completion-receipt before the sem fires).  An idle gap here resets the
    # HAM activity window and the real matmuls run at 1.2 GHz for ~3.4us
    # (measured +4.4us wall), so round UP: one extra dummy costs only 106ns.
    DUMMIES = 26

    with TileContext(nc) as tc:
        with (
            tc.tile_pool(name="wfp", bufs=8) as wfp,
            tc.tile_pool(name="pp", bufs=8, space="PSUM") as pp,
            tc.tile_pool(name="op", bufs=2) as op,
        ):
            # PE warm-up: the HAM clock gate keeps PE at 1.2 GHz until it has
            # been busy ~3.4us.  Run dummy matmuls while the first DMAs are in
            # flight so the real matmuls start at 2.4 GHz.  memset on GpSimd:
            # its preamble finishes ~0.4us before Vector's, so the first
            # dummy LDWEIGHTS issues that much earlier.
            dummy = op.tile([P, P], F16, tag="dummy", bufs=1)
            nc.gpsimd.memset(dummy, 0)
            ps_all = [
                pp.tile([P, B_LOC], mybir.dt.float32, name=f"ps{gt}", tag="ps")
                for gt in range(GT)
            ]
            for _ in range(DUMMIES):
                nc.tensor.matmul(
                    ps_all[GT - 1][:, :P], lhsT=dummy, rhs=dummy,
                    start=True, stop=True,
                )

            # stream all chunks; keep tiles of the final K_TAIL k-tiles live
            tail_tiles = []
            k0 = 0
            for ci, ck in enumerate(CHUNKS):
                wf_sb = wfp.tile([P, ck, WF], F16, tag="wf", padded_shape=[P, 4, WF])
                if ci == 0:
                    # Split chunk 0 so the first matmuls (gt0..3) wait only
                    # on feat k0 + the first half of W k0 (256KB, ready
                    # ~0.3us before the full 384KB chunk); gt4..7 wait on
                    # the rest.  Sub-DMAs keep >=1KB/partition descriptors.
                    nc.sync.dma_start(wf_sb[:, :, G:], wf_t[:, 0:ck, G:])
                    nc.sync.dma_start(wf_sb[:, :, 0 : G // 2], wf_t[:, 0:ck, 0 : G // 2])
                    nc.sync.dma_start(wf_sb[:, :, G // 2 : G], wf_t[:, 0:ck, G // 2 : G])
                else:
                    nc.sync.dma_start(wf_sb, wf_t[:, k0 : k0 + ck, :])
                if k0 >= KT - K_TAIL:
                    tail_tiles.append((k0, wf_sb))
                    k0 += ck
                    continue
                for kl in range(ck):
                    k = k0 + kl
                    for gt in range(GT):
                        nc.tensor.matmul(
                            ps_all[gt],
                            lhsT=wf_sb[:, kl, gt * P : (gt + 1) * P],
                            rhs=wf_sb[:, kl, G : G + B_LOC],
                            start=(k == 0),
                            stop=False,
                        )
                k0 += ck

            # final K_TAIL k-tiles run gt-major, so psums complete staggered
            # and each copy+store overlaps the remaining gt's matmuls.  All
            # copies ride Vector (idle otherwise); out-DMAs ride the Scalar
            # ring so they never queue behind the input stream on Sync.
            for gt in range(GT):
                for k0, wf_sb in tail_tiles:
                    for kl in range(wf_sb.shape[1]):
                        k = k0 + kl
                        nc.tensor.matmul(
                            ps_all[gt],
                            lhsT=wf_sb[:, kl, gt * P : (gt + 1) * P],
                            rhs=wf_sb[:, kl, G : G + B_LOC],
                            start=False,
                            stop=(k == KT - 1),
                        )
                if gt < GT - 1:
                    o_sb = op.tile([P, B_LOC], F32, tag="o")
                    nc.vector.tensor_copy(o_sb, ps_all[gt])
                    nc.scalar.dma_start(out_t[gt * P : (gt + 1) * P, :], o_sb)
                else:
                    # Last gt is the critical path.  Two 256-col chunks let
                    # copy c1 overlap DMA c0's issue+descriptor pipeline:
                    # measured lastMM->transfer-end 3.0us vs 3.2-3.5us for a
                    # single copy+DMA.
                    for c0 in range(0, B_LOC, 256):
                        o_cb = op.tile([P, 256], F32, tag="ocb", bufs=2)
                        nc.vector.tensor_copy(o_cb, ps_all[gt][:, c0 : c0 + 256])
                        nc.scalar.dma_start(
                            out_t[gt * P : (gt + 1) * P, c0 : c0 + 256], o_cb
                        )
    nc.compile()
    return nc


def make_in_maps(gene_set_features, wmat):
    feat = np.asarray(gene_set_features, dtype=np.float32).astype(np.float16)
    # (P, KT, G): wmat_tiled[p, k, g] = W[k*128 + p, g]
    w_tiled = wmat.astype(np.float16).reshape(KT, P, G).transpose(1, 0, 2)
    in_maps = []
    for c in range(N_CORES):
        shard = feat[c * B_LOC : (c + 1) * B_LOC, :]  # (B_LOC, NG)
        # (P, KT, B_LOC): feat_tiled[p, k, b] = shard[b, k*128 + p]
        feat_tiled = shard.T.reshape(KT, P, B_LOC).transpose(1, 0, 2)
        # fused (P, KT, G + B_LOC): W block then feat block per k-tile, so a
        # single in-order DMA stream supplies both operands k-lockstep.
        wf = np.concatenate([w_tiled, feat_tiled], axis=2)
        in_maps.append({"wf": np.ascontiguousarray(wf)})
    return in_maps


def kernel(gene_set_features, att_weights, flat_idx, segment_ids, num_segments):
    from concourse.bass_utils import run_bass_kernel_spmd

    wmat = _host_softmax_scatter(att_weights, flat_idx, segment_ids, num_segments)
    in_maps = make_in_maps(gene_set_features, wmat)

    if "nc" not in _NC_CACHE:
        _NC_CACHE["nc"] = build_nc()
    nc = _NC_CACHE["nc"]

    res = run_bass_kernel_spmd(nc, in_maps, core_ids=list(range(N_CORES)))

    out = np.empty((B, G), dtype=np.float32)
    for c in range(N_CORES):
        out[c * B_LOC : (c + 1) * B_LOC, :] = res.results[c]["outT"].T
    return out

